# revision 19
# baseline (speedup 1.0000x reference)
"""Causal multi-head attention block on 8 trn2 NeuronCores.

Problem (hardcoded): x [4, 2048, 1024] fp32, W_attn [1024, 3072], W_proj
[1024, 1024]; H=16 heads, D=64; scores scaled by 1/sqrt(1024); causal
softmax; y @ W_proj.

Sharding: core c -> (batch b = c//2, head-group hg = c%2 of 8 heads).
Each core computes q,k,v for its batch + head-group, causal attention,
and a partial projection out_partial = y_slice @ W_proj[rows of its
head-group].  Host sums the two partials per batch.

Device-side layout trick: everything is computed transposed (d on
partitions, tokens on the free axis) so no on-device transposes are
needed:
  qT/kT = W_slice.T @ xT          (xT passed pre-transposed from host)
  sT[j,i] = k_j . q_i             (lhsT = kT tile, rhs = qT range)
  ET = exp(sT/32)                 (no max-subtraction: |s/32| < ~1.5 by
                                   construction of the input distribution)
  yT_un[d,i], Z[i] = v_aug.T @ ET (v_aug has an all-ones 65th column, so
                                   row 64 of the product is the softmax
                                   denominator -- free on the PE)
  out = (yT_un/Z).T @ W_proj_slice

v2 scheduling: the attention inner loop is exp-bound on the Scalar
engine (the PE has ~40% slack there), while the QKV projections and the
output projection are pure-PE stretches.  A credit-based pump
interleaves QKV-chunk(r+1) and proj(r-1) matmuls into attention(r)'s
jt loop so the PE fills its exp-wait gaps instead of stalling, and the
Scalar engine never starves.  PSUM: pss 2x[128,1024] (4 banks) for
scores, py 2x[128,512] (2) for EV accum, pq 2x[128,512] (2) shared by
QKV/proj/final-broadcast chains.
"""

import os
from contextlib import ExitStack

import numpy as np
import ml_dtypes

import concourse.bass as bass
import concourse.mybir as mybir
from concourse import bacc, tile
from concourse.bass_utils import run_bass_kernel_spmd

B, L, C, H, D = 4, 2048, 1024, 16, 64
P = 128
NCORES = 8
NH = 8          # heads per core
NPAIR = 4       # head pairs per core
CK = C // P     # 8 contraction k-tiles over C
NCH = 4         # 512-token chunks per batch
NR = 4          # query i-ranges of 512
NJT = 16        # key j-tiles of 128
BF16 = mybir.dt.bfloat16
F32 = mybir.dt.float32

_COMPILED = None

# ns cost model for the filler credit pump (measured on hw, steady state)
_EXP_NS_PER_COL = 1.07      # scalar exp, fp32 psum in
_MM_ONE = 260.0             # one K=128 N=512 filler matmul


def _build_program(reps=1):
    nc = bacc.Bacc("TRN2", target_bir_lowering=False, debug=False,
                   num_devices=NCORES)
    xT_d = nc.dram_tensor("xt", [C, L], BF16, kind="ExternalInput")
    wq_d = nc.dram_tensor("wq", [C, 512], BF16, kind="ExternalInput")
    wk_d = nc.dram_tensor("wk", [C, 512], BF16, kind="ExternalInput")
    wv_d = nc.dram_tensor("wv", [C, 512], BF16, kind="ExternalInput")
    wp_d = nc.dram_tensor("wp", [512, C], BF16, kind="ExternalInput")
    mk_d = nc.dram_tensor("mk", [P, 2048], BF16, kind="ExternalInput")
    out_d = nc.dram_tensor("out", [L, C], F32, kind="ExternalOutput")

    with tile.TileContext(nc) as tc, ExitStack() as ctx:
        const = ctx.enter_context(tc.tile_pool(name="const", bufs=1))
        etp = ctx.enter_context(tc.tile_pool(name="et", bufs=4))
        ysp = ctx.enter_context(tc.tile_pool(name="ys", bufs=16))
        zgp = ctx.enter_context(tc.tile_pool(name="zg", bufs=5))
        z0p = ctx.enter_context(tc.tile_pool(name="z0", bufs=3))
        zbp = ctx.enter_context(tc.tile_pool(name="zb", bufs=2))
        ytp = ctx.enter_context(tc.tile_pool(name="ytmp", bufs=4))
        op = ctx.enter_context(tc.tile_pool(name="ob", bufs=2))
        pss = ctx.enter_context(
            tc.tile_pool(name="pss", bufs=2, space=bass.MemorySpace.PSUM))
        py = ctx.enter_context(
            tc.tile_pool(name="py", bufs=2, space=bass.MemorySpace.PSUM))
        pq = ctx.enter_context(
            tc.tile_pool(name="pq", bufs=2, space=bass.MemorySpace.PSUM))

        xT = const.tile([P, CK, L], BF16)
        wq = const.tile([P, CK, 512], BF16)
        wk = const.tile([P, CK, 512], BF16)
        wv = const.tile([P, CK, 512], BF16)
        wp = const.tile([P, NPAIR, C], BF16)
        mk = const.tile([P, 2048], BF16)
        qT = const.tile([P, NPAIR, L], BF16)
        kT = const.tile([P, NPAIR, L], BF16)
        vsb = const.tile([P, NH, NJT, 65], BF16)
        yT = const.tile([P, NPAIR, L], BF16)

        # DMA in consumption order: the v chains for chunk 0 run first
        # (attention pair 0 needs all of vsb jts 0-3), then the q/k
        # chains.  Interleave per-k slices so the first matmul can start
        # as soon as the first few slices land.
        xT_v = xT_d.ap().rearrange("(k p) n -> p k n", p=P)
        wv_v = wv_d.ap().rearrange("(k p) n -> p k n", p=P)
        wq_v = wq_d.ap().rearrange("(k p) n -> p k n", p=P)
        for k in range(CK):
            nc.sync.dma_start(wv[:, k, :], wv_v[:, k, :])
            nc.sync.dma_start(xT[:, k, 0:512], xT_v[:, k, 0:512])
        for k in range(CK):
            nc.sync.dma_start(wq[:, k, :], wq_v[:, k, :])
        nc.sync.dma_start(wk[:], wk_d.ap().rearrange("(k p) n -> p k n", p=P))
        nc.sync.dma_start(mk[:], mk_d.ap())
        for ch in range(1, NCH):
            cs = slice(ch * 512, (ch + 1) * 512)
            for k in range(CK):
                nc.sync.dma_start(xT[:, k, cs], xT_v[:, k, cs])
        nc.sync.dma_start(wp[:], wp_d.ap().rearrange("(k p) n -> p k n", p=P))
        nc.vector.memset(vsb[:, :, :, 64:65], 1.0)

        for _rep in range(reps):
            _phase12(nc, pss, py, pq, etp, ysp, zgp, z0p, zbp, ytp, op,
                     xT, wq, wk, wv, wp, mk, qT, kT, vsb, yT, out_d)

    nc.compile()
    return nc


def _phase12(nc, pss, py, pq, etp, ysp, zgp, z0p, zbp, ytp, op,
             xT, wq, wk, wv, wp, mk, qT, kT, vsb, yT, out_d):
        # ---- QKV chain emitters (also packaged as filler units) ----
        def qk_units(ch, p, which):
            cs = slice(ch * 512, (ch + 1) * 512)
            w = wq if which == "q" else wk
            dst = qT if which == "q" else kT
            st = {}

            def mm(k):
                if k == 0:
                    st["t"] = pq.tile([P, 512], F32, name="psq", tag="pq")
                nc.tensor.matmul(
                    st["t"][:], w[:, k, p * P:(p + 1) * P], xT[:, k, cs],
                    start=(k == 0), stop=(k == CK - 1))

            def cp():
                nc.vector.tensor_copy(dst[:, p, cs], st["t"][:])

            return [(_MM_ONE, lambda k=k: mm(k)) for k in range(CK)] \
                + [(40.0, cp)]

        def v_units(ch, sub):
            jt = ch * 4 + sub
            st = {}

            def mm(k):
                if k == 0:
                    st["t"] = pq.tile([P, 512], F32, name="psv", tag="pq")
                nc.tensor.matmul(
                    st["t"][:], xT[:, k, jt * P:(jt + 1) * P], wv[:, k, :],
                    start=(k == 0), stop=(k == CK - 1))

            def cp():
                nc.vector.tensor_copy(
                    vsb[:, :, jt, 0:64],
                    st["t"][:].rearrange("p (h d) -> p h d", h=NH))

            return [(_MM_ONE, lambda k=k: mm(k)) for k in range(CK)] \
                + [(40.0, cp)]

        def qkv_chunk_units(ch):
            units = []
            for sub in range(4):
                units += v_units(ch, sub)
            for p in range(NPAIR):
                units += qk_units(ch, p, "q")
                units += qk_units(ch, p, "k")
            return units

        # ---- output projection for one 128-token block, as units ----
        def proj_units(r, it):
            tok = r * 512 + it * P
            st = {}

            def mm(nh, p0):
                if p0 == 0:
                    st[nh] = pq.tile([P, 512], F32, name="pph", tag="pq")
                t = st[nh]
                for p in (p0, p0 + 1):
                    nc.tensor.matmul(
                        t[:], yT[:, p, tok:tok + P],
                        wp[:, p, nh * 512:(nh + 1) * 512],
                        start=(p == 0), stop=(p == NPAIR - 1))

            def cpd(nh):
                # gpsimd cannot read PSUM; both halves go to DVE
                ob = op.tile([P, 512], F32)
                nc.vector.tensor_copy(ob[:], st[nh][:])
                nc.sync.dma_start(
                    out_d.ap()[tok:tok + P, nh * 512:(nh + 1) * 512], ob[:])

            units = []
            for nh in range(2):
                units += [(_MM_ONE, lambda nh=nh, p0=p0: mm(nh, p0))
                          for p0 in (0, 2)]
                units.append((40.0, lambda nh=nh: cpd(nh)))
            return units

        # ---- filler pump: `must` (qkv chunk r+1, deadline = range end)
        # drains before `soft` (proj r-1, deadline-free, drifts).  Queues
        # hold CHAINS (unit lists); a started chain is always finished
        # before any other chain's units run -- chains share the pq PSUM
        # pool, and interleaving two chains there can deadlock the
        # in-order engines (chain A's matmul waiting on a pool slot only
        # freed by chain B's copy that is queued behind it).
        must = []
        soft = []
        credit = [0.0]
        started = set()

        def pump(budget):
            credit[0] += budget
            while True:
                q = must if must else soft
                if not q or q[0][0][0] > credit[0]:
                    break
                chain = q[0]
                started.add(id(chain))
                cost, fn = chain.pop(0)
                fn()
                credit[0] -= cost
                if not chain:
                    q.pop(0)

        def _drain(chain):
            for _, fn in chain:
                fn()
            chain.clear()

        def finish_partial_soft():
            # called before refilling `must`: a half-done soft chain must
            # not interleave with the new must chains
            if soft and id(soft[0]) in started:
                _drain(soft[0])
                soft.pop(0)

        def flush_must():
            while must:
                _drain(must[0])
                must.pop(0)
            credit[0] = 0.0

        def flush_soft():
            while soft:
                _drain(soft[0])
                soft.pop(0)

        # ---- attention for one (range, pair): scores/exp/EV ----
        def attn_pair(r, p, zg, ysbs):
            njt = 4 * (r + 1)
            psy = [py.tile([P, 512], F32, name=f"psy{hh}", tag="py")
                   for hh in range(2)]
            prev = None
            prev_n = 0

            def emit_ev(jt, et, last):
                mj = jt - 4 * r
                nst = P * mj if mj > 0 else 0
                for hh in range(2):
                    nc.tensor.matmul(
                        psy[hh][0:65, nst:512],
                        vsb[:, 2 * p + hh, jt, :],
                        et[:, hh * 512 + nst:(hh + 1) * 512],
                        start=(jt == 0), stop=last)

            for jt in range(njt):
                m = jt - 4 * r
                nst = P * m if m >= 0 else 0  # causal-narrowed col start
                n = 512 - nst
                pss_t = pss.tile([P, 1024], F32, name="pss", tag="pss")
                for hh in range(2):
                    hs = slice(hh * 64, (hh + 1) * 64)
                    nc.tensor.matmul(
                        pss_t[:, hh * 512 + nst:(hh + 1) * 512],
                        kT[hs, p, jt * P:(jt + 1) * P],
                        qT[hs, p, r * 512 + nst:(r + 1) * 512],
                        start=True, stop=True)
                et = etp.tile([P, 1024], BF16)
                scl = float(1.0 / np.sqrt(C))
                if m < 0:
                    nc.scalar.activation(
                        et[:], pss_t[:], mybir.ActivationFunctionType.Exp,
                        scale=scl)
                else:
                    ev3 = et[:].rearrange("q (t n) -> q t n", t=2)
                    pv3 = pss_t[:].rearrange("q (t n) -> q t n", t=2)
                    nc.scalar.activation(
                        ev3[:, :, nst:], pv3[:, :, nst:],
                        mybir.ActivationFunctionType.Exp, scale=scl)
                    # only the 128-wide diagonal band needs masking
                    # (on gpsimd: DVE is the busier engine)
                    tri = mk[:, m * 512 + nst:m * 512 + nst + P]
                    for hh in range(2):
                        nc.gpsimd.tensor_mul(
                            et[:, hh * 512 + nst:hh * 512 + nst + P],
                            et[:, hh * 512 + nst:hh * 512 + nst + P],
                            tri)
                if prev is not None:
                    emit_ev(jt - 1, prev, last=False)
                # exp keeps Scalar busy 2*n*1.07ns; the PE's mandatory
                # work this step is the fused scores pair (~0.75n) + the
                # EV pair for jt-1 (~1.01*prev_n): bank the difference
                # and spend it on QKV/proj filler matmuls.
                pump(2 * n * _EXP_NS_PER_COL - 0.75 * n - 1.01 * prev_n)
                prev = et
                prev_n = n
            emit_ev(njt - 1, prev, last=True)

            # evacuate psy so the banks recycle to the next pair: one
            # [65,512] copy per head gets y and the Z row together (DVE
            # cost is free-size only); the Z rows go to the bf16 [8,512]
            # gather tile via 1-descriptor DMAs.
            pair_ysb = []
            for hh in range(2):
                ys = ysp.tile([65, 512], BF16)
                nc.vector.tensor_copy(ys[:], psy[hh][0:65, :])
                row = 2 * p + hh
                nc.sync.dma_start(zg[row:row + 1, :], ys[64:65, :])
                pair_ysb.append(ys)
            ysbs.append(pair_ysb)

        # ---- softmax normalization for (a subset of pairs of) a range.
        # 1/Z rows are replicated onto 64 partitions by pure DMA (zero
        # engine time); everything is bf16 so the muls get the DVE 2x
        # mode.
        def norm(r, ysbs, zg, pairs):
            rs = slice(r * 512, (r + 1) * 512)
            # engines require 32-aligned partition windows: run the
            # row-wise ops on all 8 rows (free-size-bound, same cost);
            # rows of pairs not yet evacuated hold garbage and are
            # simply never read downstream
            zgf = zgp.tile([8, 512], F32)
            nc.vector.tensor_copy(zgf[:], zg[:])
            rz8 = zgp.tile([8, 512], F32)
            nc.vector.reciprocal_approx_fast(rz8[:], zgf[:])
            rz8b = zgp.tile([8, 512], BF16)
            nc.vector.tensor_copy(rz8b[:], rz8[:])
            for p in pairs:
                for hh in range(2):
                    row = 2 * p + hh
                    r0 = z0p.tile([1, 512], BF16)
                    nc.sync.dma_start(r0[:], rz8b[row:row + 1, :])
                    zb = zbp.tile([64, 512], BF16)
                    nc.gpsimd.partition_broadcast(zb[:], r0[:])
                    if hh == 0:
                        nc.vector.tensor_mul(
                            yT[0:64, p, rs], ysbs[p][hh][0:64, :], zb[:])
                    else:
                        yt = ytp.tile([64, 512], BF16)
                        nc.vector.tensor_mul(yt[:], ysbs[p][hh][0:64, :], zb[:])
                        for c4 in range(4):
                            cs = slice(c4 * 128, (c4 + 1) * 128)
                            nc.sync.dma_start(
                                yT[64:128, p, r * 512 + c4 * 128:
                                   r * 512 + (c4 + 1) * 128],
                                yt[:, cs])

        # ---- schedule ----
        # chunk 0 is emitted directly (v first: attention pair 0 needs
        # all four v j-tiles; q/k pair p lands right before its
        # attention pair).  Ranges r>=1 get norm(r-1) first (DVE), then
        # attention with proj(r-1) + qkv(r+1) as fillers, flushed at
        # range end so attn(r+1)'s inputs are complete.
        state = {}
        _drain(qkv_chunk_units(0))
        for r in range(NCH):
            zg = zgp.tile([8, 512], BF16)
            ysbs = []
            if r < NCH - 1:
                finish_partial_soft()
                must.append(qkv_chunk_units(r + 1))
            last = r == NCH - 1
            for p in range(NPAIR):
                attn_pair(r, p, zg, ysbs)
                # norm(r-1) is emitted mid-range so its elementwise ops
                # queue BEHIND this range's early mask-muls/evacuations
                # on the in-order engines (at range start they'd delay
                # the diagonal EVs by several us)
                if p == 1 and r >= 1:
                    norm(r - 1, *state.pop(r - 1), pairs=range(NPAIR))
                    for it in range(4):
                        soft.append(proj_units(r - 1, it))
                # final range: normalize completed pairs early so only
                # pair 3's norm chain is exposed in the tail
                if last and p == 2:
                    norm(r, ysbs, zg, pairs=[0, 1, 2])
            state[r] = (ysbs, zg)
            flush_must()
        flush_soft()
        norm(NCH - 1, *state[NCH - 1], pairs=[3])
        for it in range(4):
            _drain(proj_units(NCH - 1, it))


def get_program(reps=1):
    global _COMPILED
    if _COMPILED is None:
        _COMPILED = _build_program(reps=reps)
    return _COMPILED


def make_in_maps(x, W_attn, W_proj):
    bf = ml_dtypes.bfloat16
    x = np.asarray(x, np.float32)
    W_attn = np.asarray(W_attn, np.float32)
    W_proj = np.asarray(W_proj, np.float32)

    # causal sub-tile masks for the 4 diagonal positions of a 512-wide
    # i-range: mask[m][j, i_local] = (i_local >= 128*m + j)
    i_loc = np.arange(512)[None, :]
    j_loc = np.arange(P)[:, None]
    mk = np.concatenate(
        [(i_loc >= P * m + j_loc) for m in range(4)], axis=1).astype(bf)

    in_maps = []
    for c in range(NCORES):
        b, hg = c // 2, c % 2
        cols = slice(hg * 512, hg * 512 + 512)
        in_maps.append({
            "xt": np.ascontiguousarray(x[b].T.astype(bf)),
            "wq": np.ascontiguousarray(W_attn[:, cols].astype(bf)),
            "wk": np.ascontiguousarray(W_attn[:, 1024:2048][:, cols].astype(bf)),
            "wv": np.ascontiguousarray(W_attn[:, 2048:3072][:, cols].astype(bf)),
            "wp": np.ascontiguousarray(W_proj[hg * 512:hg * 512 + 512, :].astype(bf)),
            "mk": mk,
        })
    return in_maps


def combine_outputs(results):
    out = np.zeros((B, L, C), np.float32)
    for c in range(NCORES):
        out[c // 2] += results[c]["out"]
    return out


def kernel(x, W_attn, W_proj):
    nc = get_program()
    in_maps = make_in_maps(x, W_attn, W_proj)
    res = run_bass_kernel_spmd(nc, in_maps, list(range(NCORES)))
    return combine_outputs(res.results)


# revision 20
# speedup vs baseline: 1.2230x; 1.2230x over previous
"""Causal multi-head attention block on 8 trn2 NeuronCores.

Problem (hardcoded): x [4, 2048, 1024] fp32, W_attn [1024, 3072], W_proj
[1024, 1024]; H=16 heads, D=64; scores scaled by 1/sqrt(1024); causal
softmax; y @ W_proj.

Sharding: core c -> (batch b = c//2, head-group hg = c%2 of 8 heads).
Each core computes q,k,v for its batch + head-group, causal attention,
and a partial projection out_partial = y_slice @ W_proj[rows of its
head-group].  Host sums the two partials per batch.

Device-side layout trick: everything is computed transposed (d on
partitions, tokens on the free axis) so no on-device transposes are
needed:
  qT/kT = W_slice.T @ xT          (xT passed pre-transposed from host)
  sT[j,i] = k_j . q_i             (lhsT = kT tile, rhs = qT range)
  ET = exp(sT/32)                 (no max-subtraction: |s/32| < ~1.5 by
                                   construction of the input distribution)
  yT_un[d,i], Z[i] = v_aug.T @ ET (v_aug has an all-ones 65th column, so
                                   row 64 of the product is the softmax
                                   denominator -- free on the PE)
  out = (yT_un/Z).T @ W_proj_slice

v2 scheduling: the attention inner loop is exp-bound on the Scalar
engine (the PE has ~40% slack there), while the QKV projections and the
output projection are pure-PE stretches.  A credit-based pump
interleaves QKV-chunk(r+1) and proj(r-1) matmuls into attention(r)'s
jt loop so the PE fills its exp-wait gaps instead of stalling, and the
Scalar engine never starves.  PSUM: pss 2x[128,1024] (4 banks) for
scores, py 2x[128,512] (2) for EV accum, pq 2x[128,512] (2) shared by
QKV/proj/final-broadcast chains.
"""

import os
from contextlib import ExitStack

import numpy as np
import ml_dtypes

import concourse.bass as bass
import concourse.mybir as mybir
from concourse import bacc, tile
from concourse.bass_utils import run_bass_kernel_spmd

B, L, C, H, D = 4, 2048, 1024, 16, 64
P = 128
NCORES = 8
NH = 8          # heads per core
NPAIR = 4       # head pairs per core
CK = C // P     # 8 contraction k-tiles over C
NCH = 4         # 512-token chunks per batch
NR = 4          # query i-ranges of 512
NJT = 16        # key j-tiles of 128
BF16 = mybir.dt.bfloat16
F32 = mybir.dt.float32

_COMPILED = None

# ns cost model for the filler credit pump (measured on hw, steady state)
_EXP_NS_PER_COL = 1.07      # scalar exp, fp32 psum in
_MM_ONE = 260.0             # one K=128 N=512 filler matmul


def _build_program(reps=1):
    nc = bacc.Bacc("TRN2", target_bir_lowering=False, debug=False,
                   num_devices=NCORES)
    xT_d = nc.dram_tensor("xt", [C, L], BF16, kind="ExternalInput")
    wq_d = nc.dram_tensor("wq", [C, 512], BF16, kind="ExternalInput")
    wk_d = nc.dram_tensor("wk", [C, 512], BF16, kind="ExternalInput")
    wv_d = nc.dram_tensor("wv", [C, 512], BF16, kind="ExternalInput")
    wp_d = nc.dram_tensor("wp", [512, C], BF16, kind="ExternalInput")
    mk_d = nc.dram_tensor("mk", [P, 2048], BF16, kind="ExternalInput")
    out_d = nc.dram_tensor("out", [L, C], F32, kind="ExternalOutput")

    with tile.TileContext(nc) as tc, ExitStack() as ctx:
        const = ctx.enter_context(tc.tile_pool(name="const", bufs=1))
        etp = ctx.enter_context(tc.tile_pool(name="et", bufs=4))
        ysp = ctx.enter_context(tc.tile_pool(name="ys", bufs=16))
        zgp = ctx.enter_context(tc.tile_pool(name="zg", bufs=5))
        z0p = ctx.enter_context(tc.tile_pool(name="z0", bufs=3))
        zbp = ctx.enter_context(tc.tile_pool(name="zb", bufs=2))
        ytp = ctx.enter_context(tc.tile_pool(name="ytmp", bufs=4))
        op = ctx.enter_context(tc.tile_pool(name="ob", bufs=2))
        pss = ctx.enter_context(
            tc.tile_pool(name="pss", bufs=2, space=bass.MemorySpace.PSUM))
        py = ctx.enter_context(
            tc.tile_pool(name="py", bufs=2, space=bass.MemorySpace.PSUM))
        pq = ctx.enter_context(
            tc.tile_pool(name="pq", bufs=2, space=bass.MemorySpace.PSUM))

        xT = const.tile([P, CK, L], BF16)
        wq = const.tile([P, CK, 512], BF16)
        wk = const.tile([P, CK, 512], BF16)
        wv = const.tile([P, CK, 512], BF16)
        wp = const.tile([P, NPAIR, C], BF16)
        mk = const.tile([P, 2048], BF16)
        qT = const.tile([P, NPAIR, L], BF16)
        kT = const.tile([P, NPAIR, L], BF16)
        vsb = const.tile([P, NH, NJT, 65], BF16)
        yT = const.tile([P, NPAIR, L], BF16)

        # DMA in consumption order: the v chains for chunk 0 run first
        # (attention pair 0 needs all of vsb jts 0-3), then the q/k
        # chains.  Interleave per-k slices so the first matmul can start
        # as soon as the first few slices land.
        xT_v = xT_d.ap().rearrange("(k p) n -> p k n", p=P)
        wv_v = wv_d.ap().rearrange("(k p) n -> p k n", p=P)
        wq_v = wq_d.ap().rearrange("(k p) n -> p k n", p=P)
        for k in range(CK):
            nc.sync.dma_start(wv[:, k, :], wv_v[:, k, :])
            nc.sync.dma_start(xT[:, k, 0:512], xT_v[:, k, 0:512])
        for k in range(CK):
            nc.sync.dma_start(wq[:, k, :], wq_v[:, k, :])
        nc.sync.dma_start(wk[:], wk_d.ap().rearrange("(k p) n -> p k n", p=P))
        nc.sync.dma_start(mk[:], mk_d.ap())
        for ch in range(1, NCH):
            cs = slice(ch * 512, (ch + 1) * 512)
            for k in range(CK):
                nc.sync.dma_start(xT[:, k, cs], xT_v[:, k, cs])
        nc.sync.dma_start(wp[:], wp_d.ap().rearrange("(k p) n -> p k n", p=P))
        nc.vector.memset(vsb[:, :, :, 64:65], 1.0)

        for _rep in range(reps):
            _phase12(nc, pss, py, pq, etp, ysp, zgp, z0p, zbp, ytp, op,
                     xT, wq, wk, wv, wp, mk, qT, kT, vsb, yT, out_d)

    nc.compile()
    return nc


def _phase12(nc, pss, py, pq, etp, ysp, zgp, z0p, zbp, ytp, op,
             xT, wq, wk, wv, wp, mk, qT, kT, vsb, yT, out_d):
        # ---- QKV chain emitters (also packaged as filler units) ----
        def qk_units(ch, p, which):
            cs = slice(ch * 512, (ch + 1) * 512)
            w = wq if which == "q" else wk
            dst = qT if which == "q" else kT
            st = {}

            def mm(k):
                if k == 0:
                    st["t"] = pq.tile([P, 512], F32, name="psq", tag="pq")
                nc.tensor.matmul(
                    st["t"][:], w[:, k, p * P:(p + 1) * P], xT[:, k, cs],
                    start=(k == 0), stop=(k == CK - 1))

            def cp():
                nc.vector.tensor_copy(dst[:, p, cs], st["t"][:])

            return [(_MM_ONE, lambda k=k: mm(k)) for k in range(CK)] \
                + [(40.0, cp)]

        def v_units(ch, sub):
            jt = ch * 4 + sub
            st = {}

            def mm(k):
                if k == 0:
                    st["t"] = pq.tile([P, 512], F32, name="psv", tag="pq")
                nc.tensor.matmul(
                    st["t"][:], xT[:, k, jt * P:(jt + 1) * P], wv[:, k, :],
                    start=(k == 0), stop=(k == CK - 1))

            def cp():
                nc.vector.tensor_copy(
                    vsb[:, :, jt, 0:64],
                    st["t"][:].rearrange("p (h d) -> p h d", h=NH))

            return [(_MM_ONE, lambda k=k: mm(k)) for k in range(CK)] \
                + [(40.0, cp)]

        def qkv_chunk_units(ch):
            units = []
            for sub in range(4):
                units += v_units(ch, sub)
            for p in range(NPAIR):
                units += qk_units(ch, p, "q")
                units += qk_units(ch, p, "k")
            return units

        # ---- output projection for one 128-token block, as units ----
        def proj_units(r, it):
            tok = r * 512 + it * P
            st = {}

            def mm(nh, p0):
                if p0 == 0:
                    st[nh] = pq.tile([P, 512], F32, name="pph", tag="pq")
                t = st[nh]
                for p in (p0, p0 + 1):
                    nc.tensor.matmul(
                        t[:], yT[:, p, tok:tok + P],
                        wp[:, p, nh * 512:(nh + 1) * 512],
                        start=(p == 0), stop=(p == NPAIR - 1))

            def cpd(nh):
                # gpsimd cannot read PSUM; both halves go to DVE
                ob = op.tile([P, 512], F32)
                nc.vector.tensor_copy(ob[:], st[nh][:])
                nc.sync.dma_start(
                    out_d.ap()[tok:tok + P, nh * 512:(nh + 1) * 512], ob[:])

            units = []
            for nh in range(2):
                units += [(_MM_ONE, lambda nh=nh, p0=p0: mm(nh, p0))
                          for p0 in (0, 2)]
                units.append((40.0, lambda nh=nh: cpd(nh)))
            return units

        # ---- filler pump: `must` (qkv chunk r+1, deadline = range end)
        # drains before `soft` (proj r-1, deadline-free, drifts).  Queues
        # hold CHAINS (unit lists); a started chain is always finished
        # before any other chain's units run -- chains share the pq PSUM
        # pool, and interleaving two chains there can deadlock the
        # in-order engines (chain A's matmul waiting on a pool slot only
        # freed by chain B's copy that is queued behind it).
        must = []
        soft = []
        credit = [0.0]
        started = set()

        def pump(budget):
            credit[0] += budget
            while True:
                q = must if must else soft
                if not q or q[0][0][0] > credit[0]:
                    break
                chain = q[0]
                started.add(id(chain))
                cost, fn = chain.pop(0)
                fn()
                credit[0] -= cost
                if not chain:
                    q.pop(0)

        def _drain(chain):
            for _, fn in chain:
                fn()
            chain.clear()

        def finish_partial_soft():
            # called before refilling `must`: a half-done soft chain must
            # not interleave with the new must chains
            if soft and id(soft[0]) in started:
                _drain(soft[0])
                soft.pop(0)

        def flush_must():
            while must:
                _drain(must[0])
                must.pop(0)
            credit[0] = 0.0

        def flush_soft():
            while soft:
                _drain(soft[0])
                soft.pop(0)

        # ---- attention for one (range, pair): scores/exp/EV ----
        def attn_pair(r, p, zg, ysbs):
            njt = 4 * (r + 1)
            psy = [py.tile([P, 512], F32, name=f"psy{hh}", tag="py")
                   for hh in range(2)]
            prev = None
            prev_n = 0

            def emit_ev(jt, et, last):
                mj = jt - 4 * r
                nst = P * mj if mj > 0 else 0
                for hh in range(2):
                    nc.tensor.matmul(
                        psy[hh][0:65, nst:512],
                        vsb[:, 2 * p + hh, jt, :],
                        et[:, hh * 512 + nst:(hh + 1) * 512],
                        start=(jt == 0), stop=last)

            for jt in range(njt):
                m = jt - 4 * r
                nst = P * m if m >= 0 else 0  # causal-narrowed col start
                n = 512 - nst
                pss_t = pss.tile([P, 1024], F32, name="pss", tag="pss")
                for hh in range(2):
                    hs = slice(hh * 64, (hh + 1) * 64)
                    nc.tensor.matmul(
                        pss_t[:, hh * 512 + nst:(hh + 1) * 512],
                        kT[hs, p, jt * P:(jt + 1) * P],
                        qT[hs, p, r * 512 + nst:(r + 1) * 512],
                        start=True, stop=True)
                et = etp.tile([P, 1024], BF16)
                scl = float(1.0 / np.sqrt(C))
                if m < 0:
                    nc.scalar.activation(
                        et[:], pss_t[:], mybir.ActivationFunctionType.Exp,
                        scale=scl)
                else:
                    ev3 = et[:].rearrange("q (t n) -> q t n", t=2)
                    pv3 = pss_t[:].rearrange("q (t n) -> q t n", t=2)
                    nc.scalar.activation(
                        ev3[:, :, nst:], pv3[:, :, nst:],
                        mybir.ActivationFunctionType.Exp, scale=scl)
                    # only the 128-wide diagonal band needs masking.
                    # NB must stay on DVE: mixing tensor ops with
                    # PartitionBroadcast on gpsimd forces ~6us ucode
                    # LIBRARY_RELOAD stalls at every switch
                    tri = mk[:, m * 512 + nst:m * 512 + nst + P]
                    for hh in range(2):
                        nc.vector.tensor_mul(
                            et[:, hh * 512 + nst:hh * 512 + nst + P],
                            et[:, hh * 512 + nst:hh * 512 + nst + P],
                            tri)
                if prev is not None:
                    emit_ev(jt - 1, prev, last=False)
                # exp keeps Scalar busy 2*n*1.07ns; the PE's mandatory
                # work this step is the fused scores pair (~0.75n) + the
                # EV pair for jt-1 (~1.01*prev_n): bank the difference
                # and spend it on QKV/proj filler matmuls.
                pump(2 * n * _EXP_NS_PER_COL - 0.75 * n - 1.01 * prev_n)
                prev = et
                prev_n = n
            emit_ev(njt - 1, prev, last=True)

            # evacuate psy so the banks recycle to the next pair: one
            # [65,512] copy per head gets y and the Z row together (DVE
            # cost is free-size only); the Z rows go to the bf16 [8,512]
            # gather tile via 1-descriptor DMAs.
            pair_ysb = []
            for hh in range(2):
                ys = ysp.tile([65, 512], BF16)
                nc.vector.tensor_copy(ys[:], psy[hh][0:65, :])
                row = 2 * p + hh
                nc.sync.dma_start(zg[row:row + 1, :], ys[64:65, :])
                pair_ysb.append(ys)
            ysbs.append(pair_ysb)

        # ---- softmax normalization for (a subset of pairs of) a range.
        # 1/Z rows are replicated onto 64 partitions by pure DMA (zero
        # engine time); everything is bf16 so the muls get the DVE 2x
        # mode.
        def norm(r, ysbs, zg, pairs):
            rs = slice(r * 512, (r + 1) * 512)
            # engines require 32-aligned partition windows: run the
            # row-wise ops on all 8 rows (free-size-bound, same cost);
            # rows of pairs not yet evacuated hold garbage and are
            # simply never read downstream
            zgf = zgp.tile([8, 512], F32)
            nc.vector.tensor_copy(zgf[:], zg[:])
            rz8 = zgp.tile([8, 512], F32)
            nc.vector.reciprocal_approx_fast(rz8[:], zgf[:])
            rz8b = zgp.tile([8, 512], BF16)
            nc.vector.tensor_copy(rz8b[:], rz8[:])
            for p in pairs:
                for hh in range(2):
                    row = 2 * p + hh
                    r0 = z0p.tile([1, 512], BF16)
                    nc.sync.dma_start(r0[:], rz8b[row:row + 1, :])
                    zb = zbp.tile([64, 512], BF16)
                    nc.gpsimd.partition_broadcast(zb[:], r0[:])
                    if hh == 0:
                        nc.vector.tensor_mul(
                            yT[0:64, p, rs], ysbs[p][hh][0:64, :], zb[:])
                    else:
                        yt = ytp.tile([64, 512], BF16)
                        nc.vector.tensor_mul(yt[:], ysbs[p][hh][0:64, :], zb[:])
                        for c4 in range(4):
                            cs = slice(c4 * 128, (c4 + 1) * 128)
                            nc.sync.dma_start(
                                yT[64:128, p, r * 512 + c4 * 128:
                                   r * 512 + (c4 + 1) * 128],
                                yt[:, cs])

        # ---- schedule ----
        # chunk 0 is emitted directly (v first: attention pair 0 needs
        # all four v j-tiles; q/k pair p lands right before its
        # attention pair).  Ranges r>=1 get norm(r-1) first (DVE), then
        # attention with proj(r-1) + qkv(r+1) as fillers, flushed at
        # range end so attn(r+1)'s inputs are complete.
        state = {}
        _drain(qkv_chunk_units(0))
        for r in range(NCH):
            zg = zgp.tile([8, 512], BF16)
            ysbs = []
            if r < NCH - 1:
                finish_partial_soft()
                must.append(qkv_chunk_units(r + 1))
            last = r == NCH - 1
            for p in range(NPAIR):
                attn_pair(r, p, zg, ysbs)
                # norm(r-1) is emitted mid-range so its elementwise ops
                # queue BEHIND this range's early mask-muls/evacuations
                # on the in-order engines (at range start they'd delay
                # the diagonal EVs by several us)
                if p == 1 and r >= 1:
                    norm(r - 1, *state.pop(r - 1), pairs=range(NPAIR))
                    for it in range(4):
                        soft.append(proj_units(r - 1, it))
                # final range: normalize completed pairs early so only
                # pair 3's norm chain is exposed in the tail
                if last and p == 2:
                    norm(r, ysbs, zg, pairs=[0, 1, 2])
            state[r] = (ysbs, zg)
            flush_must()
        flush_soft()
        norm(NCH - 1, *state[NCH - 1], pairs=[3])
        for it in range(4):
            _drain(proj_units(NCH - 1, it))


def get_program(reps=1):
    global _COMPILED
    if _COMPILED is None:
        _COMPILED = _build_program(reps=reps)
    return _COMPILED


def make_in_maps(x, W_attn, W_proj):
    bf = ml_dtypes.bfloat16
    x = np.asarray(x, np.float32)
    W_attn = np.asarray(W_attn, np.float32)
    W_proj = np.asarray(W_proj, np.float32)

    # causal sub-tile masks for the 4 diagonal positions of a 512-wide
    # i-range: mask[m][j, i_local] = (i_local >= 128*m + j)
    i_loc = np.arange(512)[None, :]
    j_loc = np.arange(P)[:, None]
    mk = np.concatenate(
        [(i_loc >= P * m + j_loc) for m in range(4)], axis=1).astype(bf)

    in_maps = []
    for c in range(NCORES):
        b, hg = c // 2, c % 2
        cols = slice(hg * 512, hg * 512 + 512)
        in_maps.append({
            "xt": np.ascontiguousarray(x[b].T.astype(bf)),
            "wq": np.ascontiguousarray(W_attn[:, cols].astype(bf)),
            "wk": np.ascontiguousarray(W_attn[:, 1024:2048][:, cols].astype(bf)),
            "wv": np.ascontiguousarray(W_attn[:, 2048:3072][:, cols].astype(bf)),
            "wp": np.ascontiguousarray(W_proj[hg * 512:hg * 512 + 512, :].astype(bf)),
            "mk": mk,
        })
    return in_maps


def combine_outputs(results):
    out = np.zeros((B, L, C), np.float32)
    for c in range(NCORES):
        out[c // 2] += results[c]["out"]
    return out


def kernel(x, W_attn, W_proj):
    nc = get_program()
    in_maps = make_in_maps(x, W_attn, W_proj)
    res = run_bass_kernel_spmd(nc, in_maps, list(range(NCORES)))
    return combine_outputs(res.results)


# revision 21
# speedup vs baseline: 1.2262x; 1.0026x over previous
"""Causal multi-head attention block on 8 trn2 NeuronCores.

Problem (hardcoded): x [4, 2048, 1024] fp32, W_attn [1024, 3072], W_proj
[1024, 1024]; H=16 heads, D=64; scores scaled by 1/sqrt(1024); causal
softmax; y @ W_proj.

Sharding: core c -> (batch b = c//2, head-group hg = c%2 of 8 heads).
Each core computes q,k,v for its batch + head-group, causal attention,
and a partial projection out_partial = y_slice @ W_proj[rows of its
head-group].  Host sums the two partials per batch.

Device-side layout trick: everything is computed transposed (d on
partitions, tokens on the free axis) so no on-device transposes are
needed:
  qT/kT = W_slice.T @ xT          (xT passed pre-transposed from host)
  sT[j,i] = k_j . q_i             (lhsT = kT tile, rhs = qT range)
  ET = exp(sT/32)                 (no max-subtraction: |s/32| < ~1.5 by
                                   construction of the input distribution)
  yT_un[d,i], Z[i] = v_aug.T @ ET (v_aug has an all-ones 65th column, so
                                   row 64 of the product is the softmax
                                   denominator -- free on the PE)
  out = (yT_un/Z).T @ W_proj_slice

v2 scheduling: the attention inner loop is exp-bound on the Scalar
engine (the PE has ~40% slack there), while the QKV projections and the
output projection are pure-PE stretches.  A credit-based pump
interleaves QKV-chunk(r+1) and proj(r-1) matmuls into attention(r)'s
jt loop so the PE fills its exp-wait gaps instead of stalling, and the
Scalar engine never starves.  PSUM: pss 2x[128,1024] (4 banks) for
scores, py 2x[128,512] (2) for EV accum, pq 2x[128,512] (2) shared by
QKV/proj/final-broadcast chains.
"""

import os
from contextlib import ExitStack

import numpy as np
import ml_dtypes

import concourse.bass as bass
import concourse.mybir as mybir
from concourse import bacc, tile
from concourse.bass_utils import run_bass_kernel_spmd

B, L, C, H, D = 4, 2048, 1024, 16, 64
P = 128
NCORES = 8
NH = 8          # heads per core
NPAIR = 4       # head pairs per core
CK = C // P     # 8 contraction k-tiles over C
NCH = 4         # 512-token chunks per batch
NR = 4          # query i-ranges of 512
NJT = 16        # key j-tiles of 128
BF16 = mybir.dt.bfloat16
F32 = mybir.dt.float32

_COMPILED = None

# ns cost model for the filler credit pump (measured on hw, steady state)
_EXP_NS_PER_COL = 1.07      # scalar exp, fp32 psum in
_MM_ONE = 260.0             # one K=128 N=512 filler matmul


def _build_program(reps=1):
    nc = bacc.Bacc("TRN2", target_bir_lowering=False, debug=False,
                   num_devices=NCORES)
    xT_d = nc.dram_tensor("xt", [C, L], BF16, kind="ExternalInput")
    wq_d = nc.dram_tensor("wq", [C, 512], BF16, kind="ExternalInput")
    wk_d = nc.dram_tensor("wk", [C, 512], BF16, kind="ExternalInput")
    wv_d = nc.dram_tensor("wv", [C, 512], BF16, kind="ExternalInput")
    wp_d = nc.dram_tensor("wp", [512, C], BF16, kind="ExternalInput")
    mk_d = nc.dram_tensor("mk", [P, 2048], BF16, kind="ExternalInput")
    out_d = nc.dram_tensor("out", [L, C], F32, kind="ExternalOutput")

    with tile.TileContext(nc) as tc, ExitStack() as ctx:
        const = ctx.enter_context(tc.tile_pool(name="const", bufs=1))
        etp = ctx.enter_context(tc.tile_pool(name="et", bufs=4))
        ysp = ctx.enter_context(tc.tile_pool(name="ys", bufs=16))
        zgp = ctx.enter_context(tc.tile_pool(name="zg", bufs=5))
        z0p = ctx.enter_context(tc.tile_pool(name="z0", bufs=3))
        zbp = ctx.enter_context(tc.tile_pool(name="zb", bufs=2))
        ytp = ctx.enter_context(tc.tile_pool(name="ytmp", bufs=4))
        op = ctx.enter_context(tc.tile_pool(name="ob", bufs=2))
        pss = ctx.enter_context(
            tc.tile_pool(name="pss", bufs=2, space=bass.MemorySpace.PSUM))
        py = ctx.enter_context(
            tc.tile_pool(name="py", bufs=2, space=bass.MemorySpace.PSUM))
        pq = ctx.enter_context(
            tc.tile_pool(name="pq", bufs=2, space=bass.MemorySpace.PSUM))

        xT = const.tile([P, CK, L], BF16)
        wq = const.tile([P, CK, 512], BF16)
        wk = const.tile([P, CK, 512], BF16)
        wv = const.tile([P, CK, 512], BF16)
        wp = const.tile([P, NPAIR, C], BF16)
        mk = const.tile([P, 2048], BF16)
        qT = const.tile([P, NPAIR, L], BF16)
        kT = const.tile([P, NPAIR, L], BF16)
        vsb = const.tile([P, NH, NJT, 65], BF16)
        yT = const.tile([P, NPAIR, L], BF16)

        # DMA in consumption order: the v chains for chunk 0 run first
        # (attention pair 0 needs all of vsb jts 0-3), then the q/k
        # chains.  Interleave per-k slices so the first matmul can start
        # as soon as the first few slices land.
        xT_v = xT_d.ap().rearrange("(k p) n -> p k n", p=P)
        wv_v = wv_d.ap().rearrange("(k p) n -> p k n", p=P)
        wq_v = wq_d.ap().rearrange("(k p) n -> p k n", p=P)
        for k in range(CK):
            nc.sync.dma_start(wv[:, k, :], wv_v[:, k, :])
            nc.sync.dma_start(xT[:, k, 0:512], xT_v[:, k, 0:512])
        for k in range(CK):
            nc.sync.dma_start(wq[:, k, :], wq_v[:, k, :])
        nc.sync.dma_start(wk[:], wk_d.ap().rearrange("(k p) n -> p k n", p=P))
        nc.sync.dma_start(mk[:], mk_d.ap())
        for ch in range(1, NCH):
            cs = slice(ch * 512, (ch + 1) * 512)
            for k in range(CK):
                nc.sync.dma_start(xT[:, k, cs], xT_v[:, k, cs])
        nc.sync.dma_start(wp[:], wp_d.ap().rearrange("(k p) n -> p k n", p=P))
        nc.vector.memset(vsb[:, :, :, 64:65], 1.0)

        for _rep in range(reps):
            _phase12(nc, pss, py, pq, etp, ysp, zgp, z0p, zbp, ytp, op,
                     xT, wq, wk, wv, wp, mk, qT, kT, vsb, yT, out_d)

    nc.compile()
    return nc


def _phase12(nc, pss, py, pq, etp, ysp, zgp, z0p, zbp, ytp, op,
             xT, wq, wk, wv, wp, mk, qT, kT, vsb, yT, out_d):
        # ---- QKV chain emitters (also packaged as filler units) ----
        def qk_units(ch, p, which):
            cs = slice(ch * 512, (ch + 1) * 512)
            w = wq if which == "q" else wk
            dst = qT if which == "q" else kT
            st = {}

            def mm(k):
                if k == 0:
                    st["t"] = pq.tile([P, 512], F32, name="psq", tag="pq")
                nc.tensor.matmul(
                    st["t"][:], w[:, k, p * P:(p + 1) * P], xT[:, k, cs],
                    start=(k == 0), stop=(k == CK - 1))

            def cp():
                nc.vector.tensor_copy(dst[:, p, cs], st["t"][:])

            return [(_MM_ONE, lambda k=k: mm(k)) for k in range(CK)] \
                + [(40.0, cp)]

        def v_units(ch, sub):
            jt = ch * 4 + sub
            st = {}

            def mm(k):
                if k == 0:
                    st["t"] = pq.tile([P, 512], F32, name="psv", tag="pq")
                nc.tensor.matmul(
                    st["t"][:], xT[:, k, jt * P:(jt + 1) * P], wv[:, k, :],
                    start=(k == 0), stop=(k == CK - 1))

            def cp():
                nc.vector.tensor_copy(
                    vsb[:, :, jt, 0:64],
                    st["t"][:].rearrange("p (h d) -> p h d", h=NH))

            return [(_MM_ONE, lambda k=k: mm(k)) for k in range(CK)] \
                + [(40.0, cp)]

        def qkv_chunk_units(ch):
            units = []
            for sub in range(4):
                units += v_units(ch, sub)
            for p in range(NPAIR):
                units += qk_units(ch, p, "q")
                units += qk_units(ch, p, "k")
            return units

        # ---- output projection for one 128-token block, as units ----
        def proj_units(r, it):
            tok = r * 512 + it * P
            st = {}

            def mm(nh, p0):
                if p0 == 0:
                    st[nh] = pq.tile([P, 512], F32, name="pph", tag="pq")
                t = st[nh]
                for p in (p0, p0 + 1):
                    nc.tensor.matmul(
                        t[:], yT[:, p, tok:tok + P],
                        wp[:, p, nh * 512:(nh + 1) * 512],
                        start=(p == 0), stop=(p == NPAIR - 1))

            def cpd(nh):
                # gpsimd cannot read PSUM; both halves go to DVE
                ob = op.tile([P, 512], F32)
                nc.vector.tensor_copy(ob[:], st[nh][:])
                nc.sync.dma_start(
                    out_d.ap()[tok:tok + P, nh * 512:(nh + 1) * 512], ob[:])

            units = []
            for nh in range(2):
                units += [(_MM_ONE, lambda nh=nh, p0=p0: mm(nh, p0))
                          for p0 in (0, 2)]
                units.append((40.0, lambda nh=nh: cpd(nh)))
            return units

        # ---- filler pump: `must` (qkv chunk r+1, deadline = range end)
        # drains before `soft` (proj r-1, deadline-free, drifts).  Queues
        # hold CHAINS (unit lists); a started chain is always finished
        # before any other chain's units run -- chains share the pq PSUM
        # pool, and interleaving two chains there can deadlock the
        # in-order engines (chain A's matmul waiting on a pool slot only
        # freed by chain B's copy that is queued behind it).
        must = []
        soft = []
        credit = [0.0]
        started = set()

        def pump(budget):
            credit[0] += budget
            while True:
                q = must if must else soft
                if not q or q[0][0][0] > credit[0]:
                    break
                chain = q[0]
                started.add(id(chain))
                cost, fn = chain.pop(0)
                fn()
                credit[0] -= cost
                if not chain:
                    q.pop(0)

        def _drain(chain):
            for _, fn in chain:
                fn()
            chain.clear()

        def finish_partial_soft():
            # called before refilling `must`: a half-done soft chain must
            # not interleave with the new must chains
            if soft and id(soft[0]) in started:
                _drain(soft[0])
                soft.pop(0)

        def flush_must():
            while must:
                _drain(must[0])
                must.pop(0)
            credit[0] = 0.0

        def flush_soft():
            while soft:
                _drain(soft[0])
                soft.pop(0)

        # ---- attention for one (range, pair): scores/exp/EV ----
        def attn_pair(r, p, zg, ysbs):
            njt = 4 * (r + 1)
            psy = [py.tile([P, 512], F32, name=f"psy{hh}", tag="py")
                   for hh in range(2)]
            prev = None
            prev_n = 0

            def emit_ev(jt, et, last):
                mj = jt - 4 * r
                nst = P * mj if mj > 0 else 0
                for hh in range(2):
                    nc.tensor.matmul(
                        psy[hh][0:65, nst:512],
                        vsb[:, 2 * p + hh, jt, :],
                        et[:, hh * 512 + nst:(hh + 1) * 512],
                        start=(jt == 0), stop=last)

            for jt in range(njt):
                m = jt - 4 * r
                nst = P * m if m >= 0 else 0  # causal-narrowed col start
                n = 512 - nst
                pss_t = pss.tile([P, 1024], F32, name="pss", tag="pss")
                for hh in range(2):
                    hs = slice(hh * 64, (hh + 1) * 64)
                    nc.tensor.matmul(
                        pss_t[:, hh * 512 + nst:(hh + 1) * 512],
                        kT[hs, p, jt * P:(jt + 1) * P],
                        qT[hs, p, r * 512 + nst:(r + 1) * 512],
                        start=True, stop=True)
                et = etp.tile([P, 1024], BF16)
                scl = float(1.0 / np.sqrt(C))
                if m < 0:
                    nc.scalar.activation(
                        et[:], pss_t[:], mybir.ActivationFunctionType.Exp,
                        scale=scl)
                else:
                    ev3 = et[:].rearrange("q (t n) -> q t n", t=2)
                    pv3 = pss_t[:].rearrange("q (t n) -> q t n", t=2)
                    nc.scalar.activation(
                        ev3[:, :, nst:], pv3[:, :, nst:],
                        mybir.ActivationFunctionType.Exp, scale=scl)
                    # only the 128-wide diagonal band needs masking.
                    # NB must stay on DVE: mixing tensor ops with
                    # PartitionBroadcast on gpsimd forces ~6us ucode
                    # LIBRARY_RELOAD stalls at every switch
                    tri = mk[:, m * 512 + nst:m * 512 + nst + P]
                    for hh in range(2):
                        nc.vector.tensor_mul(
                            et[:, hh * 512 + nst:hh * 512 + nst + P],
                            et[:, hh * 512 + nst:hh * 512 + nst + P],
                            tri)
                if prev is not None:
                    emit_ev(jt - 1, prev, last=False)
                # exp keeps Scalar busy 2*n*1.07ns; the PE's mandatory
                # work this step is the fused scores pair (~0.75n) + the
                # EV pair for jt-1 (~1.01*prev_n): bank the difference
                # and spend it on QKV/proj filler matmuls.
                pump(2 * n * _EXP_NS_PER_COL - 0.44 * n - 0.84 * prev_n)
                prev = et
                prev_n = n
            emit_ev(njt - 1, prev, last=True)

            # evacuate psy so the banks recycle to the next pair: one
            # [65,512] copy per head gets y and the Z row together (DVE
            # cost is free-size only); the Z rows go to the bf16 [8,512]
            # gather tile via 1-descriptor DMAs.
            pair_ysb = []
            for hh in range(2):
                ys = ysp.tile([65, 512], BF16)
                nc.vector.tensor_copy(ys[:], psy[hh][0:65, :])
                row = 2 * p + hh
                nc.sync.dma_start(zg[row:row + 1, :], ys[64:65, :])
                pair_ysb.append(ys)
            ysbs.append(pair_ysb)

        # ---- softmax normalization for (a subset of pairs of) a range.
        # 1/Z rows are replicated onto 64 partitions by pure DMA (zero
        # engine time); everything is bf16 so the muls get the DVE 2x
        # mode.
        def norm(r, ysbs, zg, pairs):
            rs = slice(r * 512, (r + 1) * 512)
            # engines require 32-aligned partition windows: run the
            # row-wise ops on all 8 rows (free-size-bound, same cost);
            # rows of pairs not yet evacuated hold garbage and are
            # simply never read downstream
            zgf = zgp.tile([8, 512], F32)
            nc.vector.tensor_copy(zgf[:], zg[:])
            rz8 = zgp.tile([8, 512], F32)
            nc.vector.reciprocal_approx_fast(rz8[:], zgf[:])
            rz8b = zgp.tile([8, 512], BF16)
            nc.vector.tensor_copy(rz8b[:], rz8[:])
            for p in pairs:
                for hh in range(2):
                    row = 2 * p + hh
                    r0 = z0p.tile([1, 512], BF16)
                    nc.sync.dma_start(r0[:], rz8b[row:row + 1, :])
                    zb = zbp.tile([64, 512], BF16)
                    nc.gpsimd.partition_broadcast(zb[:], r0[:])
                    if hh == 0:
                        nc.vector.tensor_mul(
                            yT[0:64, p, rs], ysbs[p][hh][0:64, :], zb[:])
                    else:
                        yt = ytp.tile([64, 512], BF16)
                        nc.vector.tensor_mul(yt[:], ysbs[p][hh][0:64, :], zb[:])
                        for c4 in range(4):
                            cs = slice(c4 * 128, (c4 + 1) * 128)
                            nc.sync.dma_start(
                                yT[64:128, p, r * 512 + c4 * 128:
                                   r * 512 + (c4 + 1) * 128],
                                yt[:, cs])

        # ---- schedule ----
        # chunk 0 is emitted directly (v first: attention pair 0 needs
        # all four v j-tiles; q/k pair p lands right before its
        # attention pair).  Ranges r>=1 get norm(r-1) first (DVE), then
        # attention with proj(r-1) + qkv(r+1) as fillers, flushed at
        # range end so attn(r+1)'s inputs are complete.
        state = {}
        _drain(qkv_chunk_units(0))
        for r in range(NCH):
            zg = zgp.tile([8, 512], BF16)
            ysbs = []
            if r < NCH - 1:
                finish_partial_soft()
                must.append(qkv_chunk_units(r + 1))
            last = r == NCH - 1
            for p in range(NPAIR):
                attn_pair(r, p, zg, ysbs)
                # norm(r-1) is emitted mid-range so its elementwise ops
                # queue BEHIND this range's early mask-muls/evacuations
                # on the in-order engines (at range start they'd delay
                # the diagonal EVs by several us)
                if p == 1 and r >= 1:
                    norm(r - 1, *state.pop(r - 1), pairs=range(NPAIR))
                    for it in range(4):
                        soft.append(proj_units(r - 1, it))
                # final range: normalize completed pairs early so only
                # pair 3's norm chain is exposed in the tail
                if last and p == 2:
                    norm(r, ysbs, zg, pairs=[0, 1, 2])
            state[r] = (ysbs, zg)
            flush_must()
        # pair-3 norm first: its DVE/gpsimd/DMA chain overlaps the
        # remaining proj flush on the PE
        norm(NCH - 1, *state[NCH - 1], pairs=[3])
        flush_soft()
        for it in range(4):
            _drain(proj_units(NCH - 1, it))


def get_program(reps=1):
    global _COMPILED
    if _COMPILED is None:
        _COMPILED = _build_program(reps=reps)
    return _COMPILED


def make_in_maps(x, W_attn, W_proj):
    bf = ml_dtypes.bfloat16
    x = np.asarray(x, np.float32)
    W_attn = np.asarray(W_attn, np.float32)
    W_proj = np.asarray(W_proj, np.float32)

    # causal sub-tile masks for the 4 diagonal positions of a 512-wide
    # i-range: mask[m][j, i_local] = (i_local >= 128*m + j)
    i_loc = np.arange(512)[None, :]
    j_loc = np.arange(P)[:, None]
    mk = np.concatenate(
        [(i_loc >= P * m + j_loc) for m in range(4)], axis=1).astype(bf)

    in_maps = []
    for c in range(NCORES):
        b, hg = c // 2, c % 2
        cols = slice(hg * 512, hg * 512 + 512)
        in_maps.append({
            "xt": np.ascontiguousarray(x[b].T.astype(bf)),
            "wq": np.ascontiguousarray(W_attn[:, cols].astype(bf)),
            "wk": np.ascontiguousarray(W_attn[:, 1024:2048][:, cols].astype(bf)),
            "wv": np.ascontiguousarray(W_attn[:, 2048:3072][:, cols].astype(bf)),
            "wp": np.ascontiguousarray(W_proj[hg * 512:hg * 512 + 512, :].astype(bf)),
            "mk": mk,
        })
    return in_maps


def combine_outputs(results):
    out = np.zeros((B, L, C), np.float32)
    for c in range(NCORES):
        out[c // 2] += results[c]["out"]
    return out


def kernel(x, W_attn, W_proj):
    nc = get_program()
    in_maps = make_in_maps(x, W_attn, W_proj)
    res = run_bass_kernel_spmd(nc, in_maps, list(range(NCORES)))
    return combine_outputs(res.results)
